# revision 1
# baseline (speedup 1.0000x reference)
"""Trainium2 Bass kernel for a 2-layer GATv2 + global-mean-pool + MLP network.

Strategy (8 NeuronCores, SPMD):
  - Nodes are partitioned contiguously: core c owns nodes [6250c, 6250(c+1)).
  - Edges (incl. self-loops) are grouped by destination core; each core
    processes the edges whose dst it owns; segment softmax/sums stay local.
  - The layer-1 source-feature table (xl1 = x @ wl1 + bl1) is computed
    replicated on every core (cheap matmul), so layer-1 needs no exchange.
  - For layer 2 each core computes xl2/xr2 for its own nodes and the rows are
    AllGathered (small: [6272, 64] f32 per core).
  - Per-edge gathers use the dma_gather (InstDMAGatherAnt) custom DMA op.
  - Segment softmax-denominators and weighted message sums are computed with
    one-hot indicator matmuls accumulated in PSUM (edges on partitions).
  - Pooled graph sums are AllReduced; every core runs the tiny MLP.

Host-side work is restricted to integer index manipulation (edge partitioning,
sorting, padding, index remapping) plus dtype casts/layout; all floating-point
compute runs on the NeuronCores.
"""
import numpy as np
import ml_dtypes

import concourse.bacc as bacc
import concourse.mybir as mybir
import concourse.tile as tile
from concourse.bass_utils import run_bass_kernel_spmd

# problem constants (hardcoded per task contract)
N = 50000
IN = 64
HID = 16
HEADS = 8
OUT = 32
B = 256
NEG = 0.2
NCORES = 8
P = 128
F1 = HEADS * HID          # 128
NLOC = N // NCORES        # 6250
NBLK = (NLOC + P - 1) // P  # 49
NPAD = NBLK * P           # 6272
NXPAD = ((N + P - 1) // P) * P  # 50176
HALF1 = 25000             # src-half boundary for int16 gather indices (L1 table)
HALF2 = NPAD * (NCORES // 2)  # 25088 (L2 table)
SUPER = 4                 # edge sub-tiles per supertile

f32 = mybir.dt.float32
bf16 = mybir.dt.bfloat16
i16 = mybir.dt.int16
bfnp = ml_dtypes.bfloat16

DEBUG = False
PHASE_LIMIT = 99
REPS = 1
ABLATE = set()
GATHER_SP = False
GATHER_CHUNK = 0
NQ = 2
PB_BUFS = 2  # 0 = whole-range single gather


# ----------------------------------------------------------------------------
# host-side schedule construction (integer metadata only)
# ----------------------------------------------------------------------------

def _wrap_idx16(vals):
    """dma_gather int16 index layout: value j at [16g + j%16, j//16], g=0..7."""
    a = vals.reshape(-1, 16).T.astype(np.int16)   # [16, n/16]
    return np.tile(a, (8, 1))


def build_schedule(edge_index, batch):
    src = np.concatenate([edge_index[0].astype(np.int64), np.arange(N, dtype=np.int64)])
    dst = np.concatenate([edge_index[1].astype(np.int64), np.arange(N, dtype=np.int64)])
    core = dst // NLOC
    d_loc = dst - core * NLOC
    blk = d_loc // P
    drel = d_loc % P
    grp = (src >= HALF1).astype(np.int64)

    key_cbg = (core * NBLK + blk) * 2 + grp
    cnt = np.bincount(key_cbg, minlength=NCORES * NBLK * 2).reshape(NCORES, NBLK, 2)
    tiles = -(-cnt // P)
    T = tiles.max(axis=0)                      # [NBLK, 2] shared schedule
    T[:, 1] += (-T.sum(axis=1)) % SUPER        # pad block total to x SUPER
    Tb = T.sum(axis=1)
    T_tot = int(Tb.sum())
    ET = T_tot * P

    seg_sizes = (T * P).reshape(-1)            # [NBLK*2] edges per (blk,grp) seg
    seg_off = np.concatenate([[0], np.cumsum(seg_sizes)])[:-1]
    seg_grp = np.repeat(np.arange(NBLK * 2) % 2, seg_sizes)  # [ET]

    per_core = []
    for c in range(NCORES):
        m = core == c
        s_c, d_c, r_c, g_c = src[m], d_loc[m], drel[m], grp[m]
        key = blk[m] * 2 + g_c
        order = np.argsort(key, kind="stable")
        s_c, d_c, r_c, key = s_c[order], d_c[order], r_c[order], key[order]
        cnt_c = np.bincount(key, minlength=NBLK * 2)
        within = np.arange(len(key)) - np.concatenate([[0], np.cumsum(cnt_c)])[:-1][key]
        pos = seg_off[key] + within

        srcA = seg_grp * HALF1                 # pad: row 0 of the half's table
        srcA = srcA.astype(np.int64)
        drelA = np.full(ET, 999.0, np.float32)
        dstA = np.zeros(ET, np.int64)
        srcA[pos] = s_c
        dstA[pos] = d_c
        drelA[pos] = r_c

        idx1 = srcA - seg_grp * HALF1
        sc_core = srcA // NLOC
        idx2 = (NPAD * sc_core + (srcA - sc_core * NLOC)) - seg_grp * HALF2
        assert idx1.min() >= 0 and idx1.max() < 32768
        assert idx2.min() >= 0 and idx2.max() < 32768
        assert dstA.min() >= 0 and dstA.max() < NPAD

        g0 = np.full(NPAD, 999.0, np.float32)
        g0[:NLOC] = batch[c * NLOC:(c + 1) * NLOC].astype(np.float32)
        bra = np.where(g0 < P, g0, 999.0).astype(np.float32)
        brb = np.where((g0 >= P) & (g0 < 2 * P), g0 - P, 999.0).astype(np.float32)

        per_core.append(dict(
            srci1=_wrap_idx16(idx1),
            srci2=_wrap_idx16(idx2),
            dsti=_wrap_idx16(dstA),
            drel=np.ascontiguousarray(drelA.reshape(T_tot, P).T).astype(bfnp),
            batchrelA=np.ascontiguousarray(bra.reshape(NBLK, P).T),
            batchrelB=np.ascontiguousarray(brb.reshape(NBLK, P).T),
        ))

    return dict(T=T, Tb=Tb, T_tot=T_tot, ET=ET), per_core


# ----------------------------------------------------------------------------
# device program
# ----------------------------------------------------------------------------

def _elu(nc, pool, out_ap, in_ap, shape, tag):
    """out = ELU(in) = relu(x) + exp(-relu(-x)) - 1   (f32)."""
    rn = pool.tile(shape, f32, tag=tag + "_rn")
    nc.scalar.activation(rn[:], in_ap, mybir.ActivationFunctionType.Relu, scale=-1.0)
    en = pool.tile(shape, f32, tag=tag + "_en")
    nc.scalar.activation(en[:], rn[:], mybir.ActivationFunctionType.Exp, scale=-1.0)
    rp = pool.tile(shape, f32, tag=tag + "_rp")
    nc.scalar.activation(rp[:], in_ap, mybir.ActivationFunctionType.Relu)
    nc.vector.tensor_tensor(out_ap, rp[:], en[:], op=mybir.AluOpType.add)
    nc.vector.tensor_scalar_add(out_ap, out_ap, -1.0)




def _gather(nc, out_tile, t0, t1, in_ap, idx_sb, c0_tiles, elem, q=0):
    """Gather rows for tiles [t0, t1) of out_tile from in_ap.
    idx slice columns are 8 per tile starting at (c0_tiles + t0)."""
    ntiles = t1 - t0
    if ntiles <= 0:
        return
    chunk_t = ntiles if GATHER_CHUNK == 0 else max(1, GATHER_CHUNK // P)
    for s in range(t0, t1, chunk_t):
        e = min(s + chunk_t, t1)
        ni = (e - s) * P
        nc.gpsimd.dma_gather(
            out_ap=out_tile[:, s:e, :], in_ap=in_ap,
            idxs_ap=idx_sb[:, (c0_tiles + s) * 8:(c0_tiles + e) * 8],
            num_idxs=ni, num_idxs_reg=ni, elem_size=elem,
            single_packet=GATHER_SP, queue_num=q % NQ)

def build_program(meta):
    T = meta["T"]
    Tb = meta["Tb"]
    T_tot = meta["T_tot"]
    ET = meta["ET"]
    CI = ET // 16
    AluOp = mybir.AluOpType
    Act = mybir.ActivationFunctionType

    nc = bacc.Bacc("TRN2", target_bir_lowering=False, debug=False,
                   num_devices=NCORES, num_swdge_queues=NQ)

    def inp(name, shape, dt):
        return nc.dram_tensor(name, list(shape), dt, kind="ExternalInput")

    xpad = inp("xpad", [NXPAD, IN], f32)
    xloc = inp("xloc", [NPAD, IN], f32)
    wlr1 = inp("wlr1", [IN, 2 * F1], f32)
    biasLR1 = inp("biasLR1", [P, 2 * F1], f32)
    wlr2 = inp("wlr2", [F1, 2 * HID], f32)
    biasLR2 = inp("biasLR2", [P, 2 * HID], f32)
    bias1B = inp("bias1B", [P, F1], f32)
    bias2B = inp("bias2B", [P, HID], f32)
    attB = inp("attB", [P, F1], bf16)
    att2B = inp("att2B", [P, HID], f32)
    iota4 = inp("iota4", [P, SUPER * P], bf16)
    iotaf = inp("iotaf", [P, P], f32)
    ident = inp("ident", [P, P], f32)
    wm1 = inp("wm1", [HID, 2 * HID], f32)
    wm2 = inp("wm2", [2 * HID, HID], f32)
    wm3 = inp("wm3", [HID, OUT], f32)
    bm1B = inp("bm1B", [P, 2 * HID], f32)
    bm2B = inp("bm2B", [P, HID], f32)
    bm3B = inp("bm3B", [P, OUT], f32)
    cnt2 = inp("cnt2", [P, 2], f32)
    srci1 = inp("srci1", [P, CI], i16)
    srci2 = inp("srci2", [P, CI], i16)
    dsti = inp("dsti", [P, CI], i16)
    drel = inp("drel", [P, T_tot], bf16)
    batchrelA = inp("batchrelA", [P, NBLK], f32)
    batchrelB = inp("batchrelB", [P, NBLK], f32)

    out_ext = nc.dram_tensor("out", [B, OUT], f32, kind="ExternalOutput")
    if DEBUG:
        dbg_xlr2 = nc.dram_tensor("dbg_xlr2", [NCORES * NPAD, 4 * HID], f32,
                                  kind="ExternalOutput")
        dbg_pool = nc.dram_tensor("dbg_pool", [2 * P, HID], f32,
                                  kind="ExternalOutput")

    with nc.allow_low_precision("bf16 edge pipeline"), tile.TileContext(nc) as tc:
        with (
            tc.tile_pool(name="dram", bufs=1, space="DRAM") as dram,
            tc.tile_pool(name="const", bufs=1) as cp,
        ):
            xl1_t = dram.tile([NXPAD, F1], bf16)
            xr1_loc = dram.tile([NPAD, F1], bf16)
            xlr2_loc = dram.tile([NPAD, 4 * HID], f32)
            xlr2_all = dram.tile([NCORES * NPAD, 4 * HID], f32)
            pool_loc = dram.tile([2 * P, HID], f32)
            pool_all = dram.tile([2 * P, HID], f32)

            def const_tile(src_t, shape, dt):
                t = cp.tile(shape, dt, tag="c_" + src_t.name)
                nc.sync.dma_start(t[:], src_t[:])
                return t

            wlr1_sb = const_tile(wlr1, [IN, 2 * F1], f32)
            biasLR1_sb = const_tile(biasLR1, [P, 2 * F1], f32)
            wlr2_sb = const_tile(wlr2, [F1, 2 * HID], f32)
            biasLR2_sb = const_tile(biasLR2, [P, 2 * HID], f32)
            bias1B_sb = const_tile(bias1B, [P, F1], f32)
            bias2B_sb = const_tile(bias2B, [P, HID], f32)
            attB_sb = const_tile(attB, [P, F1], bf16)
            att2B_sb = const_tile(att2B, [P, HID], f32)
            iota4_sb = const_tile(iota4, [P, SUPER * P], bf16)
            iotaf_sb = const_tile(iotaf, [P, P], f32)
            ident_sb = const_tile(ident, [P, P], f32)
            batchrelA_sb = const_tile(batchrelA, [P, NBLK], f32)
            batchrelB_sb = const_tile(batchrelB, [P, NBLK], f32)
            poolA_acc = cp.tile([P, HID], f32)
            nc.vector.memset(poolA_acc[:], 0.0)
            poolB_acc = cp.tile([P, HID], f32)
            nc.vector.memset(poolB_acc[:], 0.0)

            for _rep in range(REPS):
                # ----------------------------------------------------------------
                # Phase A: xl1 (all nodes, bf16) and xr1 (local nodes, bf16)
                # ----------------------------------------------------------------
                with (
                    tc.tile_pool(name="pa", bufs=4) as pa,
                    tc.tile_pool(name="pa_ps", bufs=3, space="PSUM") as pa_ps,
                ):
                    def transform(x_dram, n0, rhs_ap, width, out_dram_rows):
                        xt = pa.tile([P, IN], f32, tag="xt")
                        nc.sync.dma_start(xt[:], x_dram[n0:n0 + P, :])
                        ps_t = pa_ps.tile([IN, P], f32, tag="ps_t", space="PSUM")
                        nc.tensor.transpose(out=ps_t[:], in_=xt[:], identity=ident_sb[:])
                        xT = pa.tile([IN, P], f32, tag="xT")
                        nc.scalar.copy(xT[:], ps_t[:])
                        ps_h = pa_ps.tile([P, 2 * F1], f32, tag="ps_h", space="PSUM")
                        nc.tensor.matmul(out=ps_h[:, :width], lhsT=xT[:], rhs=rhs_ap,
                                         start=True, stop=True)
                        ob = pa.tile([P, 2 * F1], bf16, tag="ob")
                        nc.vector.tensor_tensor(ob[:, :width], ps_h[:, :width],
                                                biasLR1_sb[:, :width], op=AluOp.add)
                        nc.sync.dma_start(out_dram_rows, ob[:, :width])

                    for ti in range(NXPAD // P):
                        transform(xpad, ti * P, wlr1_sb[:, 0:F1], F1,
                                  xl1_t[ti * P:(ti + 1) * P, :])
                    for ti in range(NBLK):
                        transform(xloc, ti * P, wlr1_sb[:, F1:2 * F1], F1,
                                  xr1_loc[ti * P:(ti + 1) * P, :])

                # ----------------------------------------------------------------
                # Phase B: layer-1 edge processing per dst block
                # ----------------------------------------------------------------
                C = HID  # channels per head
                with (
                    tc.tile_pool(name="pb", bufs=PB_BUFS) as pb,
                    tc.tile_pool(name="pb_fin", bufs=2) as pf,
                    tc.tile_pool(name="pb_ps", bufs=2, space="PSUM") as pb_ps,
                ):
                    ecum = 0   # edges consumed so far
                    tcum = 0   # tiles consumed so far
                    for b in range(NBLK if PHASE_LIMIT >= 2 else 0):
                        T0, T1 = int(T[b, 0]), int(T[b, 1])
                        Tt = T0 + T1
                        NEB = Tt * P
                        c16_0 = ecum // 16

                        si = pb.tile([P, NEB // 16], i16, tag="si")
                        nc.sync.dma_start(si[:], srci1[:, c16_0:c16_0 + NEB // 16])
                        di = pb.tile([P, NEB // 16], i16, tag="di")
                        nc.sync.dma_start(di[:], dsti[:, c16_0:c16_0 + NEB // 16])
                        dr = pb.tile([P, Tt], bf16, tag="dr")
                        nc.sync.dma_start(dr[:], drel[:, tcum:tcum + Tt])

                        XLg = pb.tile([P, Tt, F1], bf16, tag="XLg")
                        XRg = pb.tile([P, Tt, F1], bf16, tag="XRg")
                        if "gather" not in ABLATE:
                            _gather(nc, XLg, 0, T0, xl1_t[0:NXPAD, :], si, 0, F1)
                            _gather(nc, XLg, T0, Tt, xl1_t[HALF1:NXPAD, :], si, 0, F1)
                            _gather(nc, XRg, 0, Tt, xr1_loc[:, :], di, 0, F1, q=1)
                        else:
                            nc.vector.memset(XLg[:, 0:1, 0:8], 0.0)
                            nc.vector.memset(XRg[:, 0:1, 0:8], 0.0)

                        ps_blk = pb_ps.tile([P, HEADS + F1], f32, tag="ps_blk",
                                            space="PSUM")
                        nmm = Tt
                        mm_i = 0
                        for s0 in range(0, Tt, SUPER):
                            W = min(SUPER, Tt - s0)
                            sl = slice(s0, s0 + W)
                            S = pb.tile([P, SUPER, F1], bf16, tag="S")
                            if "dve" not in ABLATE:
                                nc.vector.tensor_tensor(S[:, :W, :], XLg[:, sl, :],
                                                        XRg[:, sl, :], op=AluOp.add)
                            if "dve" not in ABLATE:
                                nc.scalar.activation(S[:, :W, :], S[:, :W, :], Act.Prelu,
                                                     alpha=NEG)
                            PT = pb.tile([P, SUPER, F1], bf16, tag="PT")
                            if "dve" not in ABLATE:
                                nc.vector.tensor_tensor(
                                    PT[:, :W, :], S[:, :W, :],
                                    attB_sb[:].unsqueeze(1).to_broadcast([P, W, F1]),
                                    op=AluOp.mult)
                            lg = pb.tile([P, SUPER, HEADS], f32, tag="lg")
                            if "dve" not in ABLATE:
                                nc.vector.tensor_reduce(
                                    lg[:, :W, :],
                                    PT[:, :W, :].rearrange("p t (h c) -> p t h c", c=C),
                                    axis=mybir.AxisListType.X, op=AluOp.add)
                            rhs = pb.tile([P, SUPER, HEADS + F1], bf16, tag="rhs")
                            if "dve" not in ABLATE:
                                nc.scalar.activation(rhs[:, :W, 0:HEADS], lg[:, :W, :],
                                                     Act.Exp)
                            if "dve" not in ABLATE:
                                nc.vector.tensor_tensor(
                                    rhs[:, :W, HEADS:].rearrange("p t (h c) -> p t h c", c=C),
                                    XLg[:, sl, :].rearrange("p t (h c) -> p t h c", c=C),
                                    rhs[:, :W, 0:HEADS].unsqueeze(3).to_broadcast(
                                        [P, W, HEADS, C]),
                                    op=AluOp.mult)
                            ind = pb.tile([P, SUPER, P], bf16, tag="ind")
                            if "dve" not in ABLATE:
                                nc.vector.tensor_tensor(
                                    ind[:, :W, :],
                                    iota4_sb[:].rearrange("p (t q) -> p t q", q=P)[:, :W, :],
                                    dr[:, sl].unsqueeze(2).to_broadcast([P, W, P]),
                                    op=AluOp.is_equal)
                            for t in range(W):
                                if "mm" not in ABLATE:
                                    nc.tensor.matmul(out=ps_blk[:], lhsT=ind[:, t, :],
                                                     rhs=rhs[:, t, :],
                                                     start=(mm_i == 0), stop=(mm_i == nmm - 1))
                                mm_i += 1

                        den = pf.tile([P, HEADS], f32, tag="den")
                        nc.vector.tensor_scalar_max(den[:], ps_blk[:, 0:HEADS], 1e-30)
                        rden = pf.tile([P, HEADS], f32, tag="rden")
                        nc.vector.reciprocal(rden[:], den[:])
                        o1 = pf.tile([P, F1], f32, tag="o1")
                        nc.vector.tensor_tensor(
                            o1[:].rearrange("p (h c) -> p h c", c=C),
                            ps_blk[:, HEADS:].rearrange("p (h c) -> p h c", c=C),
                            rden[:].unsqueeze(2).to_broadcast([P, HEADS, C]),
                            op=AluOp.mult)
                        nc.vector.tensor_tensor(o1[:], o1[:], bias1B_sb[:], op=AluOp.add)
                        h1 = pf.tile([P, F1], f32, tag="h1")
                        _elu(nc, pf, h1[:], o1[:], [P, F1], "elu1")
                        ps_h1t = pb_ps.tile([P, P], f32, tag="ps_h1t", space="PSUM")
                        nc.tensor.transpose(out=ps_h1t[:], in_=h1[:], identity=ident_sb[:])
                        h1T = pf.tile([P, P], f32, tag="h1T")
                        nc.scalar.copy(h1T[:], ps_h1t[:])
                        ps_x2 = pb_ps.tile([P, 2 * HID], f32, tag="ps_x2", space="PSUM")
                        nc.tensor.matmul(out=ps_x2[:], lhsT=h1T[:], rhs=wlr2_sb[:],
                                         start=True, stop=True)
                        x2 = pf.tile([P, 2 * HID], f32, tag="x2")
                        nc.vector.tensor_tensor(x2[:], ps_x2[:], biasLR2_sb[:],
                                                op=AluOp.add)
                        nc.sync.dma_start(
                            xlr2_loc[b * P:(b + 1) * P, 0:2 * HID], x2[:])

                        ecum += NEB
                        tcum += Tt

                # ----------------------------------------------------------------
                # Phase C: AllGather xlr2
                # ----------------------------------------------------------------
                if PHASE_LIMIT >= 3:
                    nc.gpsimd.collective_compute(
                        "AllGather", mybir.AluOpType.bypass,
                        replica_groups=[list(range(NCORES))],
                        ins=[xlr2_loc.opt()], outs=[xlr2_all.opt()])
                if DEBUG:
                    nc.sync.dma_start(dbg_xlr2[:], xlr2_all[:])

                # ----------------------------------------------------------------
                # Phase D: layer-2 edge processing + pooling per dst block
                # ----------------------------------------------------------------
                with (
                    tc.tile_pool(name="pd", bufs=PB_BUFS) as pd,
                    tc.tile_pool(name="pd_fin", bufs=2) as pg,
                    tc.tile_pool(name="pd_ps", bufs=2, space="PSUM") as pd_ps,
                ):
                    ecum = 0
                    tcum = 0
                    for b in range(NBLK if PHASE_LIMIT >= 4 else 0):
                        T0, T1 = int(T[b, 0]), int(T[b, 1])
                        Tt = T0 + T1
                        NEB = Tt * P
                        c16_0 = ecum // 16

                        si = pd.tile([P, NEB // 16], i16, tag="si2")
                        nc.sync.dma_start(si[:], srci2[:, c16_0:c16_0 + NEB // 16])
                        di = pd.tile([P, NEB // 16], i16, tag="di2")
                        nc.sync.dma_start(di[:], dsti[:, c16_0:c16_0 + NEB // 16])
                        dr = pd.tile([P, Tt], bf16, tag="dr2")
                        nc.sync.dma_start(dr[:], drel[:, tcum:tcum + Tt])

                        XLg = pd.tile([P, Tt, 4 * HID], f32, tag="XL2")
                        XRg = pd.tile([P, Tt, 4 * HID], f32, tag="XR2")
                        if "gather" not in ABLATE:
                            _gather(nc, XLg, 0, T0, xlr2_all[0:NCORES * NPAD, :], si, 0, 4 * HID)
                            _gather(nc, XLg, T0, Tt, xlr2_all[HALF2:NCORES * NPAD, :], si, 0, 4 * HID)
                            _gather(nc, XRg, 0, Tt, xlr2_loc[:, :], di, 0, 4 * HID, q=1)
                        else:
                            nc.vector.memset(XLg[:, 0:1, 0:8], 0.0)
                            nc.vector.memset(XRg[:, 0:1, 0:8], 0.0)

                        ps_blk = pd_ps.tile([P, 1 + HID], f32, tag="ps_blk2",
                                            space="PSUM")
                        nmm = Tt
                        mm_i = 0
                        for s0 in range(0, Tt, SUPER):
                            W = min(SUPER, Tt - s0)
                            sl = slice(s0, s0 + W)
                            S = pd.tile([P, SUPER, HID], f32, tag="S2")
                            if "dve" not in ABLATE:
                                nc.vector.tensor_tensor(S[:, :W, :], XLg[:, sl, 0:HID],
                                                        XRg[:, sl, HID:2 * HID],
                                                        op=AluOp.add)
                            if "dve" not in ABLATE:
                                nc.scalar.activation(S[:, :W, :], S[:, :W, :], Act.Prelu,
                                                     alpha=NEG)
                            if "dve" not in ABLATE:
                                nc.vector.tensor_tensor(
                                    S[:, :W, :], S[:, :W, :],
                                    att2B_sb[:].unsqueeze(1).to_broadcast([P, W, HID]),
                                    op=AluOp.mult)
                            a2 = pd.tile([P, SUPER, 1], f32, tag="a2")
                            if "dve" not in ABLATE:
                                nc.vector.tensor_reduce(a2[:, :W, :], S[:, :W, :],
                                                        axis=mybir.AxisListType.X,
                                                        op=AluOp.add)
                            if "dve" not in ABLATE:
                                nc.scalar.activation(a2[:, :W, :], a2[:, :W, :], Act.Exp)
                            rhs = pd.tile([P, SUPER, 1 + HID], bf16, tag="rhs2")
                            if "dve" not in ABLATE:
                                nc.vector.tensor_copy(rhs[:, :W, 0:1], a2[:, :W, :])
                            if "dve" not in ABLATE:
                                nc.vector.tensor_tensor(
                                    rhs[:, :W, 1:], XLg[:, sl, 0:HID],
                                    a2[:, :W, :].to_broadcast([P, W, HID]),
                                    op=AluOp.mult)
                            ind = pd.tile([P, SUPER, P], bf16, tag="ind2")
                            if "dve" not in ABLATE:
                                nc.vector.tensor_tensor(
                                    ind[:, :W, :],
                                    iota4_sb[:].rearrange("p (t q) -> p t q", q=P)[:, :W, :],
                                    dr[:, sl].unsqueeze(2).to_broadcast([P, W, P]),
                                    op=AluOp.is_equal)
                            for t in range(W):
                                if "mm" not in ABLATE:
                                    nc.tensor.matmul(out=ps_blk[:], lhsT=ind[:, t, :],
                                                     rhs=rhs[:, t, :],
                                                     start=(mm_i == 0), stop=(mm_i == nmm - 1))
                                mm_i += 1

                        den = pg.tile([P, 1], f32, tag="den2")
                        nc.vector.tensor_scalar_max(den[:], ps_blk[:, 0:1], 1e-30)
                        rden = pg.tile([P, 1], f32, tag="rden2")
                        nc.vector.reciprocal(rden[:], den[:])
                        o2 = pg.tile([P, HID], f32, tag="o2")
                        nc.vector.tensor_scalar(o2[:], ps_blk[:, 1:], rden[:], None,
                                                op0=AluOp.mult)
                        nc.vector.tensor_tensor(o2[:], o2[:], bias2B_sb[:], op=AluOp.add)
                        h2 = pg.tile([P, HID], f32, tag="h2")
                        _elu(nc, pg, h2[:], o2[:], [P, HID], "elu2")

                        # pooling: per-graph partial sums via indicator matmuls
                        for half, (brsb, acc) in enumerate(
                                [(batchrelA_sb, poolA_acc), (batchrelB_sb, poolB_acc)]):
                            indp = pg.tile([P, P], f32, tag=f"indp{half}")
                            nc.vector.tensor_tensor(
                                indp[:], iotaf_sb[:],
                                brsb[:, b:b + 1].to_broadcast([P, P]),
                                op=AluOp.is_equal)
                            ps_pool = pd_ps.tile([P, HID], f32, tag=f"ps_pool{half}",
                                                 space="PSUM")
                            nc.tensor.matmul(out=ps_pool[:], lhsT=indp[:], rhs=h2[:],
                                             start=True, stop=True)
                            nc.vector.tensor_tensor(acc[:], acc[:], ps_pool[:],
                                                    op=AluOp.add)
                        ecum += NEB
                        tcum += Tt

                # ----------------------------------------------------------------
                # Phase E: AllReduce pooled sums; mean; MLP
                # ----------------------------------------------------------------
                if PHASE_LIMIT >= 5:
                    with (
                        tc.tile_pool(name="pe", bufs=2) as pe,
                        tc.tile_pool(name="pe_ps", bufs=1, space="PSUM") as pe_ps,
                    ):
                        nc.sync.dma_start(pool_loc[0:P, :], poolA_acc[:])
                        nc.sync.dma_start(pool_loc[P:2 * P, :], poolB_acc[:])
                        nc.gpsimd.collective_compute(
                            "AllReduce", mybir.AluOpType.add,
                            replica_groups=[list(range(NCORES))],
                            ins=[pool_loc.opt()], outs=[pool_all.opt()])
                        if DEBUG:
                            nc.sync.dma_start(dbg_pool[:], pool_all[:])

                        cnt_sb = const_tile(cnt2, [P, 2], f32)
                        icnt = pe.tile([P, 2], f32)
                        nc.vector.tensor_scalar_max(icnt[:], cnt_sb[:], 1.0)
                        nc.vector.reciprocal(icnt[:], icnt[:])

                        wm1_sb = const_tile(wm1, [HID, 2 * HID], f32)
                        wm2_sb = const_tile(wm2, [2 * HID, HID], f32)
                        wm3_sb = const_tile(wm3, [HID, OUT], f32)
                        bm1B_sb = const_tile(bm1B, [P, 2 * HID], f32)
                        bm2B_sb = const_tile(bm2B, [P, HID], f32)
                        bm3B_sb = const_tile(bm3B, [P, OUT], f32)

                        mean_sb = pe.tile([P, 2, HID], f32)
                        for h in range(2):
                            ph = pe.tile([P, HID], f32, tag="ph")
                            nc.sync.dma_start(ph[:], pool_all[h * P:(h + 1) * P, :])
                            nc.vector.tensor_scalar(mean_sb[:, h, :], ph[:],
                                                    icnt[:, h:h + 1], None, op0=AluOp.mult)

                        def mlp_layer(in2, kdim, wsb, wout, bsb, do_elu, tag):
                            """in2: [P, 2, kdim] halves; returns [P, 2, wout]."""
                            tT = pe.tile([kdim, 2 * P], f32, tag=tag + "_T")
                            for h in range(2):
                                ps_t = pe_ps.tile([kdim, P], f32, tag=tag + "_psT",
                                                  space="PSUM")
                                nc.tensor.transpose(out=ps_t[:], in_=in2[:, h, 0:kdim],
                                                    identity=ident_sb[:])
                                nc.scalar.copy(tT[:, h * P:(h + 1) * P], ps_t[:])
                            o2h = pe.tile([P, 2, wout], f32, tag=tag + "_o")
                            for h in range(2):
                                ps_m = pe_ps.tile([P, wout], f32, tag=tag + "_psM",
                                                  space="PSUM")
                                nc.tensor.matmul(out=ps_m[:], lhsT=tT[:, h * P:(h + 1) * P],
                                                 rhs=wsb[:], start=True, stop=True)
                                nc.vector.tensor_tensor(o2h[:, h, :], ps_m[:],
                                                        bsb[:, 0:wout], op=AluOp.add)
                                if do_elu:
                                    _elu(nc, pe, o2h[:, h, :], o2h[:, h, :], [P, wout],
                                         tag + f"_elu{h}")
                            return o2h

                        m1 = mlp_layer(mean_sb, HID, wm1_sb, 2 * HID, bm1B_sb, True, "m1")
                        m2 = mlp_layer(m1, 2 * HID, wm2_sb, HID, bm2B_sb, True, "m2")
                        m3 = mlp_layer(m2, HID, wm3_sb, OUT, bm3B_sb, False, "m3")
                        for h in range(2):
                            nc.sync.dma_start(out_ext[h * P:(h + 1) * P, :], m3[:, h, :])

    nc.compile()
    return nc


# ----------------------------------------------------------------------------
# host entry point
# ----------------------------------------------------------------------------

def _tileP(v):
    return np.ascontiguousarray(np.tile(np.asarray(v, np.float32).reshape(1, -1),
                                        (P, 1)))


def build_in_maps(inputs, meta, per_core):
    x = np.asarray(inputs["x"], np.float32)
    batch = np.asarray(inputs["batch"])
    wl1, bl1 = inputs["wl1"], inputs["bl1"]
    wr1, br1 = inputs["wr1"], inputs["br1"]
    att1, bias1 = inputs["att1"], inputs["bias1"]
    wl2, bl2 = inputs["wl2"], inputs["bl2"]
    wr2, br2 = inputs["wr2"], inputs["br2"]
    att2, bias2 = inputs["att2"], inputs["bias2"]
    w_m1, b_m1 = inputs["w_m1"], inputs["b_m1"]
    w_m2, b_m2 = inputs["w_m2"], inputs["b_m2"]
    w_m3, b_m3 = inputs["w_m3"], inputs["b_m3"]

    xpad = np.zeros((NXPAD, IN), np.float32)
    xpad[:N] = x
    iota4 = np.tile(np.arange(P, dtype=np.float32), (P, SUPER)).astype(bfnp)
    common = dict(
        xpad=xpad,
        wlr1=np.concatenate([wl1, wr1], axis=1).astype(np.float32),
        biasLR1=_tileP(np.concatenate([bl1, br1])),
        wlr2=np.concatenate([wl2, wr2], axis=1).astype(np.float32),
        biasLR2=_tileP(np.concatenate([bl2, br2])),
        bias1B=_tileP(bias1),
        bias2B=_tileP(bias2),
        attB=_tileP(att1.reshape(-1)).astype(bfnp),
        att2B=_tileP(att2.reshape(-1)),
        iota4=iota4,
        iotaf=np.tile(np.arange(P, dtype=np.float32), (P, 1)),
        ident=np.eye(P, dtype=np.float32),
        wm1=np.asarray(w_m1, np.float32),
        wm2=np.asarray(w_m2, np.float32),
        wm3=np.asarray(w_m3, np.float32),
        bm1B=_tileP(b_m1),
        bm2B=_tileP(b_m2),
        bm3B=_tileP(b_m3),
        cnt2=np.ascontiguousarray(
            np.bincount(batch, minlength=B).astype(np.float32).reshape(2, P).T),
    )
    in_maps = []
    for c in range(NCORES):
        m = dict(common)
        xl = np.zeros((NPAD, IN), np.float32)
        lo = c * NLOC
        hi = min(N, lo + NPAD)
        xl[:hi - lo] = x[lo:hi]
        m["xloc"] = xl
        m.update(per_core[c])
        in_maps.append(m)
    return in_maps


def kernel(x, edge_index, batch,
           wl1, bl1, wr1, br1, att1, bias1,
           wl2, bl2, wr2, br2, att2, bias2,
           w_m1, b_m1, w_m2, b_m2, w_m3, b_m3):
    inputs = dict(x=x, edge_index=np.asarray(edge_index), batch=np.asarray(batch),
                  wl1=wl1, bl1=bl1, wr1=wr1, br1=br1, att1=att1, bias1=bias1,
                  wl2=wl2, bl2=bl2, wr2=wr2, br2=br2, att2=att2, bias2=bias2,
                  w_m1=w_m1, b_m1=b_m1, w_m2=w_m2, b_m2=b_m2, w_m3=w_m3, b_m3=b_m3)
    meta, per_core = build_schedule(inputs["edge_index"], inputs["batch"])
    nc = build_program(meta)
    in_maps = build_in_maps(inputs, meta, per_core)
    res = run_bass_kernel_spmd(nc, in_maps, list(range(NCORES)))
    out = res.results[0]["out"]
    if DEBUG:
        kernel.last_results = res
    return out.astype(np.float32)

